# revision 24
# baseline (speedup 1.0000x reference)
"""GCN message-passing kernel for 8 trn2 NeuronCores.

Math:  out = segment_sum(h[edge_src], edge_dst) @ W_post + b_post,
       h = data @ W_pre + b_pre.
By linearity:
       out[d] = (sum_{e: dst=d} data[src_e]) @ (W_pre @ W_post)
                + deg[d] * (b_pre @ W_post) + b_post

Sharding: dst-node shards of 12500 per core (fully independent — no
collectives).  Each core gathers bf16 data rows (256B) for the edges landing
in its shard (dma_gather, int16 indices windowed by src range), segment-sums
them with one-hot bf16 matmuls on the TensorEngine (f32 PSUM accumulation per
128-node dst block), applies the folded projection, and writes its output
shard transposed ([64, shard]); the host re-assembles.

Everything on the hot path is bf16 (data, one-hot, folded weights); PSUM
accumulation stays f32, so the total relative error is ~0.5% (well under the
2e-2 gate).

Self-contained: only numpy + concourse imports; all shapes hardcoded.
"""

from contextlib import ExitStack

import numpy as np
import ml_dtypes

import concourse.bacc as bacc
import concourse.mybir as mybir
import concourse.tile as tile
from concourse import library_config
from concourse.bass_utils import run_bass_kernel_spmd

F32 = mybir.dt.float32
BF16 = mybir.dt.bfloat16
I16 = mybir.dt.int16
NPBF16 = ml_dtypes.bfloat16


class Cfg:
    N = 100000          # nodes
    DIN = 128           # input features
    DOUT = 64           # output features
    NC = 8              # cores
    SH = 12500          # dst nodes per core
    BS = 128            # dst block size
    NB = 98             # ceil(SH/BS) blocks per core
    NW = 4              # src windows
    WS = 25000          # window size (int16-safe)
    CU = 5              # uniform chunks per (block, window) cell
    G = 6               # blocks per gather group (6 acc psum banks + 2 out)


def _derived(cfg):
    NB, G = cfg.NB, cfg.G
    group_sizes = []
    b = 0
    while b < NB:
        group_sizes.append(min(G, NB - b))
        b += G
    slots_per_cell = cfg.CU * 128
    tot_slots = cfg.NB * cfg.NW * slots_per_cell
    return group_sizes, slots_per_cell, tot_slots


def preprocess(edge_src, edge_dst, cfg=Cfg):
    """Per-core gather-index / dst-local / degree arrays (pure index math)."""
    group_sizes, spc, tot_slots = _derived(cfg)
    src = np.asarray(edge_src).astype(np.int64)
    dst = np.asarray(edge_dst).astype(np.int64)

    core = dst // cfg.SH
    loc_node = dst - core * cfg.SH
    blk = loc_node // cfg.BS
    loc = loc_node - blk * cfg.BS
    win = src // cfg.WS
    widx = src - win * cfg.WS

    # cell id (core, blk, win) and slot position inside the padded cell
    cell = (core * cfg.NB + blk) * cfg.NW + win
    order = np.argsort(cell, kind="stable")
    cell_s = cell[order]
    counts = np.bincount(cell, minlength=cfg.NC * cfg.NB * cfg.NW)
    assert counts.max() <= spc, (counts.max(), spc)
    starts = np.zeros(cfg.NC * cfg.NB * cfg.NW, np.int64)
    starts[1:] = np.cumsum(counts)[:-1]
    rank = np.arange(len(src)) - starts[cell_s]

    # cell -> slot base inside its core's slot array, laid out gather-major:
    # for g in groups: for w in windows: for b in group: [CU*128 slots]
    cell_base = np.zeros((cfg.NB, cfg.NW), np.int64)
    gather_offsets = []   # (group, win) -> (slot_base, n_slots)
    off = 0
    b0 = 0
    for gs in group_sizes:
        for w in range(cfg.NW):
            gather_offsets.append((off, gs * spc))
            for bi in range(gs):
                cell_base[b0 + bi, w] = off + bi * spc
            off += gs * spc
        b0 += gs
    assert off == tot_slots

    slot = cell_base[blk[order], win[order]] + rank  # slot within core

    idx_all = np.zeros((cfg.NC, tot_slots), np.int16)
    loc_all = np.full((cfg.NC, tot_slots), -1.0, np.float32)
    core_s = core[order]
    idx_all[core_s, slot] = widx[order].astype(np.int16)
    loc_all[core_s, slot] = loc[order].astype(np.float32)

    # wrap into DMA layouts
    idx_dram = np.zeros((cfg.NC, 128, tot_slots // 16), np.int16)
    loc_dram = np.zeros((cfg.NC, 128, tot_slots // 128), np.float32)
    for sbase, n in gather_offsets:
        lin = idx_all[:, sbase:sbase + n]                      # [NC, n]
        wrapped = lin.reshape(cfg.NC, n // 16, 16).transpose(0, 2, 1)  # [NC,16,n/16]
        idx_dram[:, :, sbase // 16: (sbase + n) // 16] = np.tile(wrapped, (1, 8, 1))
        ll = loc_all[:, sbase:sbase + n]
        loc_dram[:, :, sbase // 128: (sbase + n) // 128] = (
            ll.reshape(cfg.NC, n // 128, 128).transpose(0, 2, 1))

    # local node ln sits at block ln//128, pos ln%128 -> flat index ln
    deg_dram = np.zeros((cfg.NC, 1, cfg.NB * 128), np.float32)
    degs = np.bincount(dst, minlength=cfg.N).astype(np.float32)
    for c in range(cfg.NC):
        deg_dram[c, 0, : cfg.SH] = degs[c * cfg.SH:(c + 1) * cfg.SH]

    return idx_dram, loc_dram, deg_dram, gather_offsets, group_sizes


def build_program(cfg=Cfg, repeat=1):
    group_sizes, spc, tot_slots = _derived(cfg)
    nc = bacc.Bacc("TRN2", target_bir_lowering=False, debug=True,
                   num_swdge_queues=4)

    Cmax = cfg.G * cfg.CU   # chunks in a full gather group
    data = nc.dram_tensor("data", [cfg.N, cfg.DIN], BF16, kind="ExternalInput")
    idxs = nc.dram_tensor("idxs", [128, tot_slots // 16], I16, kind="ExternalInput")
    locs = nc.dram_tensor("locs", [128, tot_slots // 128], BF16, kind="ExternalInput")
    deg = nc.dram_tensor("deg", [1, cfg.NB * 128], BF16, kind="ExternalInput")
    # iota_dc[p, d, c] = d  (constant; packed last dim keeps DVE in 2x mode)
    iota_in = nc.dram_tensor("iota", [128, 128 * Cmax], BF16,
                             kind="ExternalInput")
    ident_in = nc.dram_tensor("ident", [128, 128], F32, kind="ExternalInput")
    wpre_in = nc.dram_tensor("wpre", [cfg.DIN, cfg.DOUT], F32, kind="ExternalInput")
    wpost_in = nc.dram_tensor("wpost", [cfg.DOUT, cfg.DOUT], F32, kind="ExternalInput")
    bpre_in = nc.dram_tensor("bpre", [cfg.DOUT, 1], F32, kind="ExternalInput")
    bpost_in = nc.dram_tensor("bpost", [1, cfg.DOUT], BF16, kind="ExternalInput")
    out = nc.dram_tensor("out", [cfg.DOUT, cfg.NB * 128], F32, kind="ExternalOutput")

    with tile.TileContext(nc) as tc, ExitStack() as stk:
        nc.gpsimd.load_library(library_config.mlp)
        with (
            tc.tile_pool(name="consts", bufs=1) as cpool,
            tc.tile_pool(name="idxp", bufs=4) as idxp,
            tc.tile_pool(name="locp", bufs=4) as locp,
            tc.tile_pool(name="msgs", bufs=3) as msgsp,
            tc.tile_pool(name="oh", bufs=3) as ohp,
            tc.tile_pool(name="accsb", bufs=3) as accsbp,
            tc.tile_pool(name="outsb", bufs=2) as outsbp,
            tc.tile_pool(name="degp", bufs=2) as degp,
        ):
            # ---- constants & folded weights ----
            iota_sb = cpool.tile([128, 128, Cmax], BF16)
            ident_sb = cpool.tile([128, 128], F32)
            wpre_sb = cpool.tile([cfg.DIN, cfg.DOUT], F32)
            wpost_sb = cpool.tile([cfg.DOUT, cfg.DOUT], F32)
            bpre_sb = cpool.tile([cfg.DOUT, 1], F32)
            bpost_sb = cpool.tile([1, cfg.DOUT], BF16)
            ones_sb = cpool.tile([1, 128], BF16)
            nc.sync.dma_start(out=iota_sb[:], in_=iota_in[:])
            nc.sync.dma_start(out=ident_sb[:], in_=ident_in[:])
            nc.sync.dma_start(out=wpre_sb[:], in_=wpre_in[:])
            nc.sync.dma_start(out=wpost_sb[:], in_=wpost_in[:])
            nc.sync.dma_start(out=bpre_sb[:], in_=bpre_in[:])
            nc.sync.dma_start(out=bpost_sb[:], in_=bpost_in[:])
            nc.vector.memset(ones_sb[:], 1.0)

            with tc.tile_pool(name="pssetup", bufs=1, space="PSUM") as pssetup:
                wpreT_ps = pssetup.tile([cfg.DOUT, cfg.DIN], F32, tag="setup")
                nc.tensor.transpose(out=wpreT_ps[:], in_=wpre_sb[:],
                                    identity=ident_sb[:])
                wpreT_sb = cpool.tile([cfg.DOUT, cfg.DIN], F32)
                nc.vector.tensor_copy(wpreT_sb[:], wpreT_ps[:])

                wcomb_ps = pssetup.tile([cfg.DIN, cfg.DOUT], F32, tag="setup")
                nc.tensor.matmul(out=wcomb_ps[:], lhsT=wpreT_sb[:],
                                 rhs=wpost_sb[:], start=True, stop=True)
                wcomb_sb = cpool.tile([cfg.DIN, cfg.DOUT], BF16)
                nc.vector.tensor_copy(wcomb_sb[:], wcomb_ps[:])

                bpw_ps = pssetup.tile([1, cfg.DOUT], F32, tag="setup")
                nc.tensor.matmul(out=bpw_ps[:], lhsT=bpre_sb[:], rhs=wpost_sb[:],
                                 start=True, stop=True)
                bpw_sb = cpool.tile([1, cfg.DOUT], BF16)
                nc.vector.tensor_copy(bpw_sb[:], bpw_ps[:])

            psacc = stk.enter_context(
                tc.tile_pool(name="psacc", bufs=6, space="PSUM"))
            psout = stk.enter_context(
                tc.tile_pool(name="psout", bufs=2, space="PSUM"))
            if repeat > 1:
                stk.enter_context(tc.For_i(0, repeat, 1))
            # ---- main loop over gather groups ----
            # Window-sequential: each window's (msgs, onehot) pair is fully
            # consumed (all blocks' chunk-matmuls) before the next window's,
            # so only ~2 window tiles are live (double buffering); the G
            # per-block PSUM accumulators stay live across the 4 windows.
            off = 0      # slot offset
            b0 = 0       # first block of group
            for gs in group_sizes:
                n = gs * spc             # slots per gather here
                C = n // 128             # chunks per gather
                deg_t = degp.tile([1, gs * 128], BF16)
                nc.sync.dma_start(out=deg_t[:],
                                  in_=deg[:, b0 * 128: (b0 + gs) * 128])
                accs = [psacc.tile([128, 128], F32, name=f"acc{b0}_{_i}", tag="acc")
                        for _i in range(gs)]
                for w in range(cfg.NW):
                    idx_t = idxp.tile([128, n // 16], I16)
                    nc.sync.dma_start(
                        out=idx_t[:], in_=idxs[:, off // 16: (off + n) // 16])
                    loc_t = locp.tile([128, C], BF16)
                    nc.sync.dma_start(
                        out=loc_t[:], in_=locs[:, off // 128: (off + n) // 128])
                    m_t = msgsp.tile([128, C, cfg.DIN], BF16)
                    # split across the 4 SWDGE queues (disjoint chunk spans of
                    # one tile, exactly the pattern HW-validated in isolation)
                    c0 = 0
                    for j in range(4):
                        span = C // 4 + (1 if j < C % 4 else 0)
                        if span == 0:
                            continue
                        nq = span * 128
                        s0 = c0 * 128
                        nc.gpsimd.dma_gather(
                            m_t[:, c0:c0 + span, :],
                            data[w * cfg.WS: (w + 1) * cfg.WS, :],
                            idx_t[:, s0 // 16: (s0 + nq) // 16],
                            nq, nq, cfg.DIN, single_packet=False,
                            queue_num=j)
                        c0 += span
                    o_t = ohp.tile([128, 128, C], BF16)
                    nc.vector.tensor_tensor(
                        out=o_t[:],
                        in0=loc_t[:].unsqueeze(1).broadcast_to([128, 128, C]),
                        in1=iota_sb[:, :, :C],
                        op=mybir.AluOpType.is_equal)
                    for bi in range(gs):
                        for cu in range(cfg.CU):
                            ch = bi * cfg.CU + cu
                            nc.tensor.matmul(
                                out=accs[bi][:],
                                lhsT=m_t[:, ch, :],
                                rhs=o_t[:, :, ch],
                                start=(w == 0 and cu == 0),
                                stop=(w == cfg.NW - 1 and cu == cfg.CU - 1))
                    off += n

                out_t = outsbp.tile([cfg.DOUT, gs * 128], F32)
                for bi in range(gs):
                    acc_sb = accsbp.tile([128, 128], BF16)
                    nc.scalar.copy(acc_sb[:], accs[bi][:])
                    outp = psout.tile([cfg.DOUT, 128], F32)
                    nc.tensor.matmul(out=outp[:], lhsT=wcomb_sb[:], rhs=acc_sb[:],
                                     start=True, stop=False)
                    nc.tensor.matmul(out=outp[:], lhsT=bpw_sb[:],
                                     rhs=deg_t[:, bi * 128:(bi + 1) * 128],
                                     start=False, stop=False)
                    nc.tensor.matmul(out=outp[:], lhsT=bpost_sb[:], rhs=ones_sb[:],
                                     start=False, stop=True)
                    nc.scalar.copy(out_t[:, bi * 128:(bi + 1) * 128], outp[:])
                nc.sync.dma_start(
                    out=out[:, b0 * 128:(b0 + gs) * 128], in_=out_t[:])
                b0 += gs
    nc.compile()
    return nc


_PROGRAM_CACHE = {}


def _get_program(cfg=Cfg, repeat=1):
    key = (cfg.N, cfg.CU, cfg.G, repeat)
    if key not in _PROGRAM_CACHE:
        _PROGRAM_CACHE[key] = build_program(cfg, repeat=repeat)
    return _PROGRAM_CACHE[key]


def make_in_maps(data, edge_src, edge_dst, W_pre, b_pre, W_post, b_post, cfg=Cfg):
    idx_dram, loc_dram, deg_dram, _, _ = preprocess(edge_src, edge_dst, cfg)
    datab = np.ascontiguousarray(
        np.asarray(data, dtype=np.float32).astype(NPBF16))
    Cmax = cfg.G * cfg.CU
    iota = np.broadcast_to(
        np.arange(128, dtype=np.float32)[:, None], (128, Cmax))
    iota = np.tile(iota.reshape(1, 128 * Cmax), (128, 1)).astype(NPBF16)
    ident = np.eye(128, dtype=np.float32)
    wpre = np.asarray(W_pre, dtype=np.float32)
    wpost = np.asarray(W_post, dtype=np.float32)
    bpre = np.asarray(b_pre, dtype=np.float32).reshape(cfg.DOUT, 1)
    bpost = np.asarray(b_post, dtype=np.float32).reshape(1, cfg.DOUT).astype(NPBF16)
    in_maps = []
    for c in range(cfg.NC):
        in_maps.append({
            "data": datab,
            "idxs": idx_dram[c],
            "locs": loc_dram[c].astype(NPBF16),
            "deg": deg_dram[c].astype(NPBF16),
            "iota": iota,
            "ident": ident,
            "wpre": wpre,
            "wpost": wpost,
            "bpre": bpre,
            "bpost": bpost,
        })
    return in_maps


def kernel(data, edge_src, edge_dst, W_pre, b_pre, W_post, b_post):
    cfg = Cfg
    nc = _get_program(cfg)
    in_maps = make_in_maps(data, edge_src, edge_dst, W_pre, b_pre, W_post,
                           b_post, cfg)
    res = run_bass_kernel_spmd(nc, in_maps, list(range(cfg.NC)), trace=False)
    out = np.empty((cfg.N, cfg.DOUT), np.float32)
    for c in range(cfg.NC):
        out[c * cfg.SH:(c + 1) * cfg.SH, :] = res.results[c]["out"][:, :cfg.SH].T
    return out
